# revision 7
# baseline (speedup 1.0000x reference)
"""MLA (Multi-Head Latent Attention) Bass kernel for 8 Trainium2 NeuronCores.

Sharding: 8 cores = 2 (batch) x 4 (head groups). Core c -> batch c//4,
group g=c%4 owning heads {2g, 2g+1, 2g+8, 2g+9} (paired h/h+8 so the
rotate-half RoPE over d_model=2048 stays core-local).

v4: the q-path latent factorization is folded ON THE HOST:
Weff = Wq_down @ Wq_up[:, cols_g] (and @ Wq_rope) is computed in fp32
numpy inside kernel() and shipped as a bf16 input, so the kernel streams
x_q straight through Weff (512 matmuls) with NO collective dependency.
Only the small latkv+krope AllGather (0.64MB, mesh regime) remains, and
it is fully hidden behind the 140us q-strip phase. This removes v2/v3's
latq AllGather, which was serialization/skew-bound and stalled the PE
for 40-60us per run.

All activations flow on-device in transposed [feature, token] layout.
Attention scores are computed in [k, q] layout; the softmax denominator
is an all-ones matmul on the PE (scores are bounded, so no max
subtraction), exp runs on the scalar engine straight out of PSUM, and
1/denom is folded into the attention-output scaling.

Each core computes a partial out^T = (attn_out_g @ Wout[rows_g]).T for
its 4 heads; the host sums the 4 partials per batch and transposes.
bout is added on-device by the g==0 cores only.
"""
import os
import sys

if "/opt/trn_rl_repo" not in sys.path:
    sys.path.insert(0, "/opt/trn_rl_repo")

import numpy as np

D_MODEL = 2048
Q_LAT = 1536
KV_LAT = 512
NUM_HEADS = 16
HD = 128
B, S = 2, 2048
SCALE = 1.0 / np.sqrt(2.0 * HD)  # 1/16

QT = 512          # query tile width (matmul free dim)
NQT = S // QT     # 4
NC_DM = D_MODEL // 128   # 16 chunks of the model dim
NC_QL = Q_LAT // 128     # 12
NC_KV = KV_LAT // 128    # 4
NKC = S // 128           # 16 key chunks

_CACHE = {}
LAST_RESULT = None


def _strip_cols(g):
    return [256 * g, 256 * g + 128, 1024 + 256 * g, 1024 + 256 * g + 128]


def _build_bass():
    from concourse import bacc, mybir
    from concourse.tile import TileContext

    f32 = mybir.dt.float32
    bf16 = mybir.dt.bfloat16
    AF = mybir.ActivationFunctionType

    nc = bacc.Bacc("TRN2", target_bir_lowering=False, debug=False, num_devices=8)

    def inp(name, shape, dt=bf16):
        return nc.dram_tensor(name, list(shape), dt, kind="ExternalInput")

    # x repacked on host per 512-token tile, per-partition contiguous
    # (16KB DMA descriptors instead of 1KB):
    # xq_t4[qt][p][c*QT+k] = xq[b][qt*QT+k, c*128+p]
    xq_t4 = inp("xq_t4", (NQT, 128, NC_DM * QT))
    xk_p = inp("xk_p", (128, NC_DM * QT))  # this core's k-tile, same layout
    wkv_down = inp("wkv_down", (NC_KV, 128, NC_DM * 128))  # [s][p=dm][c*128+f]
    wk_rope = inp("wk_rope", (128, NC_DM * 128))           # [p=dm][c*128+f]
    # host-folded Wq_down@{Wq_up,Wq_rope}[:, cols_g]: [kind][p=dm][c*512+s*128+f]
    weff = inp("weff", (2, 128, NC_DM * 512))
    wk_up = inp("wk_up", (4, 128, NC_KV * 128))            # [strip][p=lat][c*128+f]
    wv_up = inp("wv_up", (128, NC_KV * 512))               # [p=lat][c*512+f]
    wout = inp("wout", (128, 64 * 128))                    # [p][(m*4+h)*128+f]
    cos_q = inp("cos_q", (2, 128, S))                      # [block j][d][q]
    sin_q = inp("sin_q", (2, 128, S))
    cos_k = inp("cos_k", (64, QT))
    sin_k = inp("sin_k", (64, QT))
    masktri = inp("masktri", (128, 128))                   # kl <= ql triangular
    ones = inp("ones", (128, 128))
    bias = inp("bias", (128, NC_DM), f32)                  # [p][m]

    outT = nc.dram_tensor("outT", [D_MODEL, S], f32, kind="ExternalOutput")

    # dep-free warmup collective absorbs CC setup + launch skew at t=0
    warm_sh_d = nc.dram_tensor("warm_sh_d", [128, 4], bf16, kind="Internal")
    warm_g_d = nc.dram_tensor("warm_g_d", [4, 128, 4], bf16, kind="Internal")
    # AG1: latkv (s=0..3) + krope (s=4), [p, s*QT+k] contiguous
    latk_sh_d = nc.dram_tensor("latk_sh_d", [128, 5 * QT], bf16, kind="Internal")
    latk_g_d = nc.dram_tensor("latk_g_d", [4, 128, 5 * QT], bf16, kind="Internal")
    G_BATCH = [[0, 1, 2, 3], [4, 5, 6, 7]]

    xq_t4_v = xq_t4.ap().rearrange("t p (c k) -> t p c k", k=QT)
    xk_p_v = xk_p.ap().rearrange("p (c k) -> p c k", k=QT)     # [128, 16, 512]

    with TileContext(nc) as tc:
        with tc.tile_pool(name="kvres", bufs=1) as kvres, \
             tc.tile_pool(name="qnres", bufs=1) as qnres, \
             tc.tile_pool(name="xstream", bufs=3) as xstream:
            # resident tensors consumed by phase B
            kproj_sb = kvres.tile([128, 4, S], bf16)
            krope_sb = kvres.tile([128, S], bf16)
            v_sb = kvres.tile([128, NKC, 512], bf16)
            qn_sb = qnres.tile([128, 8, S], bf16)  # [2*strip + (0=proj,1=rope)]

            # ----- Phases A1+A2 interleaved: krope -> strips(qt0) ->
            # latkv+AG -> strips(qt1..3).  The latkv block sits in the
            # middle so its 2MB weight load has ~50us to arrive, and the
            # AllGather still completes long before phase A3 needs it.
            with tc.tile_pool(name="a2w", bufs=1) as a2w, \
                 tc.tile_pool(name="a2t", bufs=2) as a2t, \
                 tc.tile_pool(name="a2ps", bufs=2, space="PSUM") as a2ps:
                weffa_sb = a2w.tile([128, NC_DM, 512], bf16)
                weffb_sb = a2w.tile([128, NC_DM, 512], bf16)

                def strips(qt, xq_t):
                    q0 = qt * QT
                    cs_t = a2t.tile([128, 2, 2, QT], bf16, tag="cs", bufs=1)
                    for j in range(2):
                        nc.sync.dma_start(
                            out=cs_t[:, 0, j, :], in_=cos_q.ap()[j][:, q0:q0 + QT])
                        nc.sync.dma_start(
                            out=cs_t[:, 1, j, :], in_=sin_q.ap()[j][:, q0:q0 + QT])
                    for s in range(4):
                        ps = a2ps.tile([128, QT], f32, tag="ps")
                        for c in range(NC_DM):
                            nc.tensor.matmul(
                                ps, weffa_sb[:, c, s * 128:(s + 1) * 128],
                                xq_t[:, c, :], start=(c == 0), stop=(c == NC_DM - 1))
                        nc.scalar.copy(out=qn_sb[:, 2 * s, q0:q0 + QT], in_=ps)
                    raw = []
                    for s in range(4):
                        ps = a2ps.tile([128, QT], f32, tag="ps")
                        for c in range(NC_DM):
                            nc.tensor.matmul(
                                ps, weffb_sb[:, c, s * 128:(s + 1) * 128],
                                xq_t[:, c, :], start=(c == 0), stop=(c == NC_DM - 1))
                        rw = a2t.tile([128, QT], bf16, tag=f"raw{s}")
                        nc.scalar.copy(out=rw, in_=ps)
                        raw.append(rw)
                    for j in range(2):
                        a, b = raw[j], raw[2 + j]
                        cj = cs_t[:, 0, j, :]
                        sj = cs_t[:, 1, j, :]
                        t1 = a2t.tile([128, QT], bf16, tag=f"t1{j}")
                        t2 = a2t.tile([128, QT], bf16, tag=f"t2{j}")
                        nc.vector.tensor_mul(t1, a, cj)
                        nc.vector.tensor_mul(t2, b, sj)
                        nc.vector.tensor_sub(qn_sb[:, 2 * j + 1, q0:q0 + QT], t1, t2)
                        nc.vector.tensor_mul(t1, b, cj)
                        nc.vector.tensor_mul(t2, a, sj)
                        nc.vector.tensor_add(qn_sb[:, 2 * (2 + j) + 1, q0:q0 + QT], t1, t2)

                def load_xq(qt):
                    xq_t = xstream.tile([128, NC_DM, QT], bf16, tag="x")
                    for q in range(2):
                        nc.sync.dma_start(
                            out=xq_t[:, 8 * q:8 * (q + 1), :],
                            in_=xq_t4_v[qt][:, 8 * q:8 * (q + 1), :])
                    return xq_t

                with tc.tile_pool(name="a1s", bufs=1) as a1s, \
                     tc.tile_pool(name="a1w", bufs=2) as a1w, \
                     tc.tile_pool(name="a1st", bufs=1) as a1st, \
                     tc.tile_pool(name="a1ps", bufs=2, space="PSUM") as a1ps:
                    # PE clock pre-warm: dummy accumulation chain on garbage
                    # SBUF, no input deps, runs while the first DMAs land
                    with tc.tile_pool(name="wrm", bufs=1) as wrm, \
                         tc.tile_pool(name="wrmps", bufs=1, space="PSUM") as wrmps:
                        wsrc = wrm.tile([128, 256], bf16)
                        nc.vector.memset(wsrc, 0.0)
                        wps = wrmps.tile([128, 256], f32)
                        for i in range(20):
                            nc.tensor.matmul(wps, wsrc[:, 0:128], wsrc,
                                             start=(i == 0), stop=(i == 19))
                    nc.gpsimd.collective_compute(
                        "AllGather", mybir.AluOpType.bypass,
                        replica_groups=G_BATCH,
                        ins=[warm_sh_d.ap()], outs=[warm_g_d.ap()])

                    # --- DMA issue order == first-use order ---
                    wkr_sb = a1s.tile([128, NC_DM * 128], bf16)
                    nc.sync.dma_start(out=wkr_sb, in_=wk_rope.ap())
                    xk_t = a1s.tile([128, NC_DM, QT], bf16)
                    for q in range(4):
                        nc.sync.dma_start(
                            out=xk_t[:, 4 * q:4 * (q + 1), :],
                            in_=xk_p_v[:, 4 * q:4 * (q + 1), :])
                    cosk_sb = a1s.tile([64, QT], bf16)
                    sink_sb = a1s.tile([64, QT], bf16)
                    nc.sync.dma_start(out=cosk_sb, in_=cos_k.ap())
                    nc.sync.dma_start(out=sink_sb, in_=sin_k.ap())
                    for q in range(4):
                        nc.sync.dma_start(
                            out=weffa_sb[:, 4 * q:4 * (q + 1), :],
                            in_=weff.ap()[0].rearrange(
                                "p (c f) -> p c f", f=512)[:, 4 * q:4 * (q + 1), :])
                    xq0_t = load_xq(0)
                    for q in range(4):
                        nc.sync.dma_start(
                            out=weffb_sb[:, 4 * q:4 * (q + 1), :],
                            in_=weff.ap()[1].rearrange(
                                "p (c f) -> p c f", f=512)[:, 4 * q:4 * (q + 1), :])

                    # --- krope chain (16 mm) + rotate-half ---
                    latk_sh = a1s.tile([128, 5, QT], bf16)
                    ps = a1ps.tile([128, QT], f32, tag="ps")
                    for c in range(NC_DM):
                        nc.tensor.matmul(
                            ps, wkr_sb[:, c * 128:(c + 1) * 128],
                            xk_t[:, c, :], start=(c == 0), stop=(c == NC_DM - 1))
                    krraw = a1st.tile([128, QT], bf16, tag="krraw")
                    nc.scalar.copy(out=krraw, in_=ps)
                    krb = a1st.tile([64, QT], bf16, tag="krb")
                    nc.sync.dma_start(out=krb, in_=krraw[64:128, :])
                    t1 = a1st.tile([64, QT], bf16, tag="krt1")
                    t2 = a1st.tile([64, QT], bf16, tag="krt2")
                    nc.vector.tensor_mul(t1, krraw[0:64, :], cosk_sb)
                    nc.vector.tensor_mul(t2, krb, sink_sb)
                    nc.vector.tensor_sub(latk_sh[0:64, 4, :], t1, t2)
                    nc.vector.tensor_mul(t1, krb, cosk_sb)
                    nc.vector.tensor_mul(t2, krraw[0:64, :], sink_sb)
                    nc.vector.tensor_add(krb, t1, t2)  # krb dead; reuse as out
                    nc.sync.dma_start(out=latk_sh[64:128, 4, :], in_=krb)

                    # --- strips for qt0 while the wkv weights stream in ---
                    strips(0, xq0_t)

                    # --- latkv (weights streamed per strip) + allgather ---
                    for s in range(NC_KV):
                        wkv_t = a1w.tile([128, NC_DM * 128], bf16, tag="wkv")
                        nc.sync.dma_start(out=wkv_t, in_=wkv_down.ap()[s])
                        ps = a1ps.tile([128, QT], f32, tag="ps")
                        for c in range(NC_DM):
                            nc.tensor.matmul(
                                ps, wkv_t[:, c * 128:(c + 1) * 128],
                                xk_t[:, c, :], start=(c == 0), stop=(c == NC_DM - 1))
                        nc.scalar.copy(out=latk_sh[:, s, :], in_=ps)
                    nc.sync.dma_start(
                        out=latk_sh_d.ap().rearrange("p (s k) -> p s k", k=QT),
                        in_=latk_sh)
                    nc.gpsimd.collective_compute(
                        "AllGather", mybir.AluOpType.bypass, replica_groups=G_BATCH,
                        ins=[latk_sh_d.ap()], outs=[latk_g_d.ap()])

                # xstream bufs=3: qt3's slot is qt0's buffer (freed early),
                # so all three remaining x tiles prefetch back-to-back
                xq1_t = load_xq(1)
                xq2_t = load_xq(2)
                xq3_t = load_xq(3)
                strips(1, xq1_t)
                strips(2, xq2_t)

                # ----- Phase A3 (k_proj + V) slotted BEFORE the last strip
                # tile: gives qt3's x another ~35us of DMA slack, and the
                # gathered latkv has long arrived by now.
                with tc.tile_pool(name="a3p", bufs=1) as a3p, \
                     tc.tile_pool(name="a3ps", bufs=2, space="PSUM") as a3ps:
                    wku_sb = a3p.tile([128, 4 * NC_KV * 128], bf16)
                    for s in range(4):
                        nc.sync.dma_start(
                            out=wku_sb[:, s * NC_KV * 128:(s + 1) * NC_KV * 128],
                            in_=wk_up.ap()[s])
                    wvu_sb = a3p.tile([128, NC_KV * 512], bf16)
                    nc.sync.dma_start(out=wvu_sb, in_=wv_up.ap())
                    latkv_a = a3p.tile([128, 4, 5, QT], bf16)
                    for kt in range(4):
                        nc.sync.dma_start(
                            out=latkv_a[:, kt],
                            in_=latk_g_d.ap()[kt].rearrange("p (s k) -> p s k", k=QT))
                        nc.vector.tensor_copy(
                            out=krope_sb[:, kt * QT:(kt + 1) * QT],
                            in_=latkv_a[:, kt, 4, :])
                    for kt in range(4):
                        latkv_t = latkv_a[:, kt]
                        for s in range(4):
                            ps = a3ps.tile([128, QT], f32, tag="ps")
                            for c in range(NC_KV):
                                nc.tensor.matmul(
                                    ps, wku_sb[:, (s * NC_KV + c) * 128:(s * NC_KV + c + 1) * 128],
                                    latkv_t[:, c, :], start=(c == 0), stop=(c == NC_KV - 1))
                            # kproj evacuations on DVE, V's on ACT: splits the
                            # PSUM-drain load across both engines
                            nc.vector.tensor_copy(
                                out=kproj_sb[:, s, kt * QT:(kt + 1) * QT], in_=ps)
                        for kc in range(4):
                            ps = a3ps.tile([128, 512], f32, tag="ps")
                            for c in range(NC_KV):
                                nc.tensor.matmul(
                                    ps, latkv_t[:, c, kc * 128:(kc + 1) * 128],
                                    wvu_sb[:, c * 512:(c + 1) * 512],
                                    start=(c == 0), stop=(c == NC_KV - 1))
                            nc.scalar.copy(out=v_sb[:, kt * 4 + kc, :], in_=ps)

                strips(3, xq3_t)

            # ------------- Phase B: attention + output projection ----------
            with tc.tile_pool(name="bw", bufs=1) as bw, \
                 tc.tile_pool(name="be", bufs=4) as be, \
                 tc.tile_pool(name="ba", bufs=2) as ba, \
                 tc.tile_pool(name="bo", bufs=4) as bo, \
                 tc.tile_pool(name="bps", bufs=2, space="PSUM") as bps, \
                 tc.tile_pool(name="bpd", bufs=2, space="PSUM") as bpd, \
                 tc.tile_pool(name="bpv", bufs=2, space="PSUM") as bpv, \
                 tc.tile_pool(name="bpo", bufs=2, space="PSUM") as bpo:
                wout_sb = bw.tile([128, 64 * 128], bf16)
                nc.sync.dma_start(out=wout_sb, in_=wout.ap())
                mtri_sb = bw.tile([128, 128], bf16)
                nc.sync.dma_start(out=mtri_sb, in_=masktri.ap())
                ones_sb = bw.tile([128, 128], bf16)
                nc.sync.dma_start(out=ones_sb, in_=ones.ap())
                bias_sb = bw.tile([128, NC_DM], f32)
                nc.sync.dma_start(out=bias_sb, in_=bias.ap())

                for qt in range(NQT):
                    q0 = qt * QT
                    K = (q0 + QT) // 128  # causal: chunks 0..K-1
                    attn = ba.tile([128, 4, QT], bf16, tag="attn")
                    for h in range(4):
                        psd = bpd.tile([128, QT], f32, tag="psd")
                        psv = bpv.tile([128, QT], f32, tag="psv")
                        # software-pipelined by one chunk: psd/psv for chunk
                        # kc-1 issue after the scores of kc, so the exp has a
                        # matmul-pair of cover and the PE never waits on ACT
                        pend = None  # (ex, w0) of the previous chunk

                        def flush(last):
                            ex, w0, kc0 = pend
                            nc.tensor.matmul(
                                psd[:, w0:], ones_sb, ex[:, w0:],
                                start=(kc0 == 0), stop=last,
                                skip_group_check=True)
                            nc.tensor.matmul(
                                psv[:, w0:], v_sb[:, kc0, h * 128:(h + 1) * 128],
                                ex[:, w0:],
                                start=(kc0 == 0), stop=last,
                                skip_group_check=True)

                        for kc in range(K):
                            # diagonal chunks: queries before 128*o are fully
                            # masked -- compute only columns [w0, QT)
                            o = kc - q0 // 128
                            w0 = 128 * o if o > 0 else 0
                            pss = bps.tile([128, QT], f32, tag="pss")
                            nc.tensor.matmul(
                                pss[:, w0:], kproj_sb[:, h, kc * 128:(kc + 1) * 128],
                                qn_sb[:, 2 * h, q0 + w0:q0 + QT],
                                start=True, stop=False)
                            nc.tensor.matmul(
                                pss[:, w0:], krope_sb[:, kc * 128:(kc + 1) * 128],
                                qn_sb[:, 2 * h + 1, q0 + w0:q0 + QT],
                                start=False, stop=True)
                            if pend is not None:
                                flush(False)
                            ex = be.tile([128, QT], bf16, tag="ex")
                            nc.scalar.activation(out=ex[:, w0:], in_=pss[:, w0:],
                                                 func=AF.Exp, scale=float(SCALE))
                            if o >= 0:  # triangular mask on the 128-col band
                                nc.vector.tensor_mul(
                                    ex[:, w0:w0 + 128], ex[:, w0:w0 + 128], mtri_sb)
                            pend = (ex, w0, kc)
                        flush(True)
                        rec = be.tile([128, QT], f32, tag="rec")
                        nc.vector.reciprocal_approx_fast(out=rec, in_=psd)
                        nc.vector.tensor_mul(attn[:, h, :], psv, rec)
                    # output projection for this q tile
                    for m in range(NC_DM):
                        pso = bpo.tile([128, QT], f32, tag="pso")
                        for h in range(4):
                            nc.tensor.matmul(
                                pso, wout_sb[:, (m * 4 + h) * 128:(m * 4 + h + 1) * 128],
                                attn[:, h, :], start=(h == 0), stop=(h == 3))
                        oc = bo.tile([128, QT], f32, tag="oc")
                        # alternate evacuation engine so PSUM banks recycle
                        # fast enough to keep the PE from micro-stalling
                        if m % 2 == 0:
                            nc.vector.tensor_scalar_add(
                                oc, pso, bias_sb[:, m:m + 1])
                        else:
                            nc.scalar.activation(
                                out=oc, in_=pso, func=AF.Identity,
                                bias=bias_sb[:, m:m + 1], scale=1.0)
                        nc.sync.dma_start(
                            out=outT.ap()[m * 128:(m + 1) * 128, q0:q0 + QT], in_=oc)

    nc.finalize()
    return nc


def _host_pack(inputs):
    """Build the 8 per-core input maps from the full inputs."""
    import ml_dtypes
    bf16 = ml_dtypes.bfloat16

    xq = np.ascontiguousarray(inputs["inputs_q"], dtype=np.float32)
    xk = np.ascontiguousarray(inputs["inputs_k"], dtype=np.float32)
    Wq_down = np.asarray(inputs["Wq_down"], dtype=np.float32)
    Wkv_down = np.asarray(inputs["Wkv_down"], dtype=np.float32)
    Wq_up = np.asarray(inputs["Wq_up"], dtype=np.float32)
    Wk_up = np.asarray(inputs["Wk_up"], dtype=np.float32)
    Wv_up = np.asarray(inputs["Wv_up"], dtype=np.float32)
    Wq_rope = np.asarray(inputs["Wq_rope"], dtype=np.float32)
    Wk_rope = np.asarray(inputs["Wk_rope"], dtype=np.float32)
    Wout = np.asarray(inputs["Wout"], dtype=np.float32)
    bout = np.asarray(inputs["bout"], dtype=np.float32)

    def pack_lhs(W, n_strips, strip_starts, nchunks):
        out = np.empty((n_strips, 128, nchunks * 128), dtype=bf16)
        for s in range(n_strips):
            blk = W[:, strip_starts[s]:strip_starts[s] + 128]
            out[s] = blk.reshape(nchunks, 128, 128).transpose(1, 0, 2).reshape(128, -1).astype(bf16)
        return out

    # per-512-token-tile, per-partition-contiguous packing:
    # xq_t4[qt, p, c*QT+k] = xq[b, qt*QT+k, c*128+p]
    xq_t4 = [
        np.ascontiguousarray(
            xq[b].T.astype(bf16).reshape(NC_DM, 128, NQT, QT)
            .transpose(2, 1, 0, 3).reshape(NQT, 128, NC_DM * QT))
        for b in range(B)]
    xkT = [xk[b].T.astype(bf16) for b in range(B)]

    # host fold (fp32): full Weff for both kinds, sliced per group below
    weff_a_full = Wq_down @ Wq_up       # [D_MODEL, D_MODEL]
    weff_b_full = Wq_down @ Wq_rope     # [D_MODEL, D_MODEL]

    wkv_down_p = pack_lhs(Wkv_down, NC_KV, [128 * s for s in range(NC_KV)], NC_DM)
    wk_rope_p = pack_lhs(Wk_rope, 1, [0], NC_DM)[0]

    iq = np.arange(1024, dtype=np.float64)
    inv_q = 1.0 / (10000.0 ** (iq * 2.0 / D_MODEL))
    pos = np.arange(S, dtype=np.float64)
    ang_q = pos[:, None] * inv_q[None, :]          # [S, 1024]
    ik = np.arange(64, dtype=np.float64)
    inv_k = 1.0 / (10000.0 ** (ik * 2.0 / HD))
    ang_k = pos[:, None] * inv_k[None, :]          # [S, 64]
    cos_k_full = np.cos(ang_k).T.astype(bf16)  # [64, S]
    sin_k_full = np.sin(ang_k).T.astype(bf16)

    kl = np.arange(128)[:, None]
    ql = np.arange(128)[None, :]
    masktri = np.ascontiguousarray((kl <= ql).astype(np.float32).astype(bf16))
    ones = np.ones((128, 128), dtype=bf16)

    in_maps = []
    for c in range(8):
        b, g = divmod(c, 4)
        cols = _strip_cols(g)
        cols4 = np.concatenate([np.arange(cs, cs + 128) for cs in cols])

        # folded q-weights: [kind][p=dm within chunk][c*512 + s*128 + f]
        weff_p = np.empty((2, 128, NC_DM * 512), dtype=bf16)
        for kind, Wf in ((0, weff_a_full), (1, weff_b_full)):
            Wg = Wf[:, cols4]  # [D_MODEL, 512]
            weff_p[kind] = (
                Wg.reshape(NC_DM, 128, 512).transpose(1, 0, 2)
                .reshape(128, -1).astype(bf16))
        wk_up_p = pack_lhs(Wk_up, 4, cols, NC_KV)
        Wv_g = Wv_up[:, cols4]                      # [512, 512]
        wv_up_p = np.ascontiguousarray(
            Wv_g.reshape(NC_KV, 128, 512).transpose(1, 0, 2).reshape(128, -1).astype(bf16))
        Wout_g = Wout[cols4, :].reshape(4, 128, NC_DM, 128)   # [h][p][m][f]
        wout_p = np.ascontiguousarray(
            Wout_g.transpose(1, 2, 0, 3).reshape(128, -1).astype(bf16))
        cos_q_p = np.empty((2, 128, S), dtype=bf16)
        sin_q_p = np.empty((2, 128, S), dtype=bf16)
        for j in range(2):
            idx = 256 * g + 128 * j + np.arange(128)
            cos_q_p[j] = np.cos(ang_q[:, idx]).T.astype(bf16)
            sin_q_p[j] = np.sin(ang_q[:, idx]).T.astype(bf16)
        bias_p = (bout if g == 0 else np.zeros_like(bout)).reshape(NC_DM, 128)
        bias_p = np.ascontiguousarray(bias_p.T)     # [128, m]

        k0 = QT * g
        xk_p = np.ascontiguousarray(
            xkT[b][:, k0:k0 + QT].reshape(NC_DM, 128, QT)
            .transpose(1, 0, 2).reshape(128, NC_DM * QT))
        in_maps.append({
            "xq_t4": xq_t4[b],
            "xk_p": xk_p,
            "wkv_down": wkv_down_p, "wk_rope": wk_rope_p,
            "weff": weff_p,
            "wk_up": wk_up_p, "wv_up": wv_up_p, "wout": wout_p,
            "cos_q": cos_q_p, "sin_q": sin_q_p,
            "cos_k": np.ascontiguousarray(cos_k_full[:, k0:k0 + QT]),
            "sin_k": np.ascontiguousarray(sin_k_full[:, k0:k0 + QT]),
            "masktri": masktri, "ones": ones, "bias": bias_p,
        })
    return in_maps


def kernel(**inputs):
    global LAST_RESULT
    from concourse.bass_utils import run_bass_kernel_spmd

    if "nc" not in _CACHE:
        _CACHE["nc"] = _build_bass()
    nc = _CACHE["nc"]

    in_maps = _host_pack(inputs)
    kwargs = {}
    if os.environ.get("KERNEL_TRACE"):
        try:
            sys.path.insert(0, os.path.dirname(os.path.abspath(__file__)))
            import axon_shim
            axon_shim.install()
        except Exception:
            pass
        kwargs["trace"] = True
    res = run_bass_kernel_spmd(nc, in_maps, core_ids=list(range(8)), **kwargs)
    LAST_RESULT = res

    out = np.empty((B, S, D_MODEL), dtype=np.float32)
    for b in range(B):
        acc = res.results[4 * b]["outT"].copy()
        for g in range(1, 4):
            acc += res.results[4 * b + g]["outT"]
        out[b] = acc.T
    return out


# revision 8
# speedup vs baseline: 1.0064x; 1.0064x over previous
"""MLA (Multi-Head Latent Attention) Bass kernel for 8 Trainium2 NeuronCores.

Sharding: 8 cores = 2 (batch) x 4 (head groups). Core c -> batch c//4,
group g=c%4 owning heads {2g, 2g+1, 2g+8, 2g+9} (paired h/h+8 so the
rotate-half RoPE over d_model=2048 stays core-local).

v4: the q-path latent factorization is folded ON THE HOST:
Weff = Wq_down @ Wq_up[:, cols_g] (and @ Wq_rope) is computed in fp32
numpy inside kernel() and shipped as a bf16 input, so the kernel streams
x_q straight through Weff (512 matmuls) with NO collective dependency.
Only the small latkv+krope AllGather (0.64MB, mesh regime) remains, and
it is fully hidden behind the 140us q-strip phase. This removes v2/v3's
latq AllGather, which was serialization/skew-bound and stalled the PE
for 40-60us per run.

All activations flow on-device in transposed [feature, token] layout.
Attention scores are computed in [k, q] layout; the softmax denominator
is an all-ones matmul on the PE (scores are bounded, so no max
subtraction), exp runs on the scalar engine straight out of PSUM, and
1/denom is folded into the attention-output scaling.

Each core computes a partial out^T = (attn_out_g @ Wout[rows_g]).T for
its 4 heads; the host sums the 4 partials per batch and transposes.
bout is added on-device by the g==0 cores only.
"""
import os
import sys

if "/opt/trn_rl_repo" not in sys.path:
    sys.path.insert(0, "/opt/trn_rl_repo")

import numpy as np

D_MODEL = 2048
Q_LAT = 1536
KV_LAT = 512
NUM_HEADS = 16
HD = 128
B, S = 2, 2048
SCALE = 1.0 / np.sqrt(2.0 * HD)  # 1/16

QT = 512          # query tile width (matmul free dim)
NQT = S // QT     # 4
NC_DM = D_MODEL // 128   # 16 chunks of the model dim
NC_QL = Q_LAT // 128     # 12
NC_KV = KV_LAT // 128    # 4
NKC = S // 128           # 16 key chunks

_CACHE = {}
LAST_RESULT = None


def _strip_cols(g):
    return [256 * g, 256 * g + 128, 1024 + 256 * g, 1024 + 256 * g + 128]


def _build_bass():
    from concourse import bacc, mybir
    from concourse.tile import TileContext

    f32 = mybir.dt.float32
    bf16 = mybir.dt.bfloat16
    AF = mybir.ActivationFunctionType

    nc = bacc.Bacc("TRN2", target_bir_lowering=False, debug=False, num_devices=8)

    def inp(name, shape, dt=bf16):
        return nc.dram_tensor(name, list(shape), dt, kind="ExternalInput")

    # x repacked on host per 512-token tile, per-partition contiguous
    # (16KB DMA descriptors instead of 1KB):
    # xq_t4[qt][p][c*QT+k] = xq[b][qt*QT+k, c*128+p]
    xq_t4 = inp("xq_t4", (NQT, 128, NC_DM * QT))
    xk_p = inp("xk_p", (128, NC_DM * QT))  # this core's k-tile, same layout
    wkv_down = inp("wkv_down", (NC_KV, 128, NC_DM * 128))  # [s][p=dm][c*128+f]
    wk_rope = inp("wk_rope", (128, NC_DM * 128))           # [p=dm][c*128+f]
    # host-folded Wq_down@{Wq_up,Wq_rope}[:, cols_g]: [kind][p=dm][c*512+s*128+f]
    weff = inp("weff", (2, 128, NC_DM * 512))
    wk_up = inp("wk_up", (4, 128, NC_KV * 128))            # [strip][p=lat][c*128+f]
    wv_up = inp("wv_up", (128, NC_KV * 512))               # [p=lat][c*512+f]
    wout = inp("wout", (128, 64 * 128))                    # [p][(m*4+h)*128+f]
    cos_q = inp("cos_q", (2, 128, S))                      # [block j][d][q]
    sin_q = inp("sin_q", (2, 128, S))
    cos_k = inp("cos_k", (64, QT))
    sin_k = inp("sin_k", (64, QT))
    masktri = inp("masktri", (128, 128))                   # kl <= ql triangular
    ones = inp("ones", (128, 128))
    bias = inp("bias", (128, NC_DM), f32)                  # [p][m]

    outT = nc.dram_tensor("outT", [D_MODEL, S], f32, kind="ExternalOutput")

    # dep-free warmup collective absorbs CC setup + launch skew at t=0
    warm_sh_d = nc.dram_tensor("warm_sh_d", [128, 4], bf16, kind="Internal")
    warm_g_d = nc.dram_tensor("warm_g_d", [4, 128, 4], bf16, kind="Internal")
    # AG1: latkv (s=0..3) + krope (s=4), [p, s*QT+k] contiguous
    latk_sh_d = nc.dram_tensor("latk_sh_d", [128, 5 * QT], bf16, kind="Internal")
    latk_g_d = nc.dram_tensor("latk_g_d", [4, 128, 5 * QT], bf16, kind="Internal")
    G_BATCH = [[0, 1, 2, 3], [4, 5, 6, 7]]

    xq_t4_v = xq_t4.ap().rearrange("t p (c k) -> t p c k", k=QT)
    xk_p_v = xk_p.ap().rearrange("p (c k) -> p c k", k=QT)     # [128, 16, 512]

    with TileContext(nc) as tc:
        with tc.tile_pool(name="kvres", bufs=1) as kvres, \
             tc.tile_pool(name="qnres", bufs=1) as qnres, \
             tc.tile_pool(name="xstream", bufs=3) as xstream:
            # resident tensors consumed by phase B
            kproj_sb = kvres.tile([128, 4, S], bf16)
            krope_sb = kvres.tile([128, S], bf16)
            v_sb = kvres.tile([128, NKC, 512], bf16)
            qn_sb = qnres.tile([128, 8, S], bf16)  # [2*strip + (0=proj,1=rope)]

            # ----- Phases A1+A2 interleaved: krope -> strips(qt0) ->
            # latkv+AG -> strips(qt1..3).  The latkv block sits in the
            # middle so its 2MB weight load has ~50us to arrive, and the
            # AllGather still completes long before phase A3 needs it.
            with tc.tile_pool(name="a2w", bufs=1) as a2w, \
                 tc.tile_pool(name="a2t", bufs=2) as a2t, \
                 tc.tile_pool(name="a2ps", bufs=2, space="PSUM") as a2ps:
                weffa_sb = a2w.tile([128, NC_DM, 512], bf16)
                weffb_sb = a2w.tile([128, NC_DM, 512], bf16)

                def strips(qt, xq_t):
                    q0 = qt * QT
                    cs_t = a2t.tile([128, 2, 2, QT], bf16, tag="cs", bufs=1)
                    for j in range(2):
                        nc.sync.dma_start(
                            out=cs_t[:, 0, j, :], in_=cos_q.ap()[j][:, q0:q0 + QT])
                        nc.sync.dma_start(
                            out=cs_t[:, 1, j, :], in_=sin_q.ap()[j][:, q0:q0 + QT])
                    for s in range(4):
                        ps = a2ps.tile([128, QT], f32, tag="ps")
                        for c in range(NC_DM):
                            nc.tensor.matmul(
                                ps, weffa_sb[:, c, s * 128:(s + 1) * 128],
                                xq_t[:, c, :], start=(c == 0), stop=(c == NC_DM - 1))
                        nc.scalar.copy(out=qn_sb[:, 2 * s, q0:q0 + QT], in_=ps)
                    raw = []
                    for s in range(4):
                        ps = a2ps.tile([128, QT], f32, tag="ps")
                        for c in range(NC_DM):
                            nc.tensor.matmul(
                                ps, weffb_sb[:, c, s * 128:(s + 1) * 128],
                                xq_t[:, c, :], start=(c == 0), stop=(c == NC_DM - 1))
                        rw = a2t.tile([128, QT], bf16, tag=f"raw{s}")
                        nc.scalar.copy(out=rw, in_=ps)
                        raw.append(rw)
                    for j in range(2):
                        a, b = raw[j], raw[2 + j]
                        cj = cs_t[:, 0, j, :]
                        sj = cs_t[:, 1, j, :]
                        t1 = a2t.tile([128, QT], bf16, tag=f"t1{j}")
                        t2 = a2t.tile([128, QT], bf16, tag=f"t2{j}")
                        nc.vector.tensor_mul(t1, a, cj)
                        nc.vector.tensor_mul(t2, b, sj)
                        nc.vector.tensor_sub(qn_sb[:, 2 * j + 1, q0:q0 + QT], t1, t2)
                        nc.vector.tensor_mul(t1, b, cj)
                        nc.vector.tensor_mul(t2, a, sj)
                        nc.vector.tensor_add(qn_sb[:, 2 * (2 + j) + 1, q0:q0 + QT], t1, t2)

                def load_xq(qt):
                    xq_t = xstream.tile([128, NC_DM, QT], bf16, tag="x")
                    for q in range(2):
                        nc.sync.dma_start(
                            out=xq_t[:, 8 * q:8 * (q + 1), :],
                            in_=xq_t4_v[qt][:, 8 * q:8 * (q + 1), :])
                    return xq_t

                with tc.tile_pool(name="a1s", bufs=1) as a1s, \
                     tc.tile_pool(name="a1w", bufs=2) as a1w, \
                     tc.tile_pool(name="a1st", bufs=1) as a1st, \
                     tc.tile_pool(name="a1ps", bufs=2, space="PSUM") as a1ps:
                    # PE clock pre-warm: dummy accumulation chain on garbage
                    # SBUF, no input deps, runs while the first DMAs land
                    with tc.tile_pool(name="wrm", bufs=1) as wrm, \
                         tc.tile_pool(name="wrmps", bufs=1, space="PSUM") as wrmps:
                        wsrc = wrm.tile([128, 256], bf16)
                        nc.vector.memset(wsrc, 0.0)
                        wps = wrmps.tile([128, 256], f32)
                        for i in range(28):
                            nc.tensor.matmul(wps, wsrc[:, 0:128], wsrc,
                                             start=(i == 0), stop=(i == 27))
                    nc.gpsimd.collective_compute(
                        "AllGather", mybir.AluOpType.bypass,
                        replica_groups=G_BATCH,
                        ins=[warm_sh_d.ap()], outs=[warm_g_d.ap()])

                    # --- DMA issue order == first-use order ---
                    wkr_sb = a1s.tile([128, NC_DM * 128], bf16)
                    nc.sync.dma_start(out=wkr_sb, in_=wk_rope.ap())
                    xk_t = a1s.tile([128, NC_DM, QT], bf16)
                    for q in range(4):
                        nc.sync.dma_start(
                            out=xk_t[:, 4 * q:4 * (q + 1), :],
                            in_=xk_p_v[:, 4 * q:4 * (q + 1), :])
                    cosk_sb = a1s.tile([64, QT], bf16)
                    sink_sb = a1s.tile([64, QT], bf16)
                    nc.sync.dma_start(out=cosk_sb, in_=cos_k.ap())
                    nc.sync.dma_start(out=sink_sb, in_=sin_k.ap())
                    for q in range(4):
                        nc.sync.dma_start(
                            out=weffa_sb[:, 4 * q:4 * (q + 1), :],
                            in_=weff.ap()[0].rearrange(
                                "p (c f) -> p c f", f=512)[:, 4 * q:4 * (q + 1), :])
                    xq0_t = load_xq(0)
                    for q in range(4):
                        nc.sync.dma_start(
                            out=weffb_sb[:, 4 * q:4 * (q + 1), :],
                            in_=weff.ap()[1].rearrange(
                                "p (c f) -> p c f", f=512)[:, 4 * q:4 * (q + 1), :])

                    # --- krope chain (16 mm) + rotate-half ---
                    latk_sh = a1s.tile([128, 5, QT], bf16)
                    ps = a1ps.tile([128, QT], f32, tag="ps")
                    for c in range(NC_DM):
                        nc.tensor.matmul(
                            ps, wkr_sb[:, c * 128:(c + 1) * 128],
                            xk_t[:, c, :], start=(c == 0), stop=(c == NC_DM - 1))
                    krraw = a1st.tile([128, QT], bf16, tag="krraw")
                    nc.scalar.copy(out=krraw, in_=ps)
                    krb = a1st.tile([64, QT], bf16, tag="krb")
                    nc.sync.dma_start(out=krb, in_=krraw[64:128, :])
                    t1 = a1st.tile([64, QT], bf16, tag="krt1")
                    t2 = a1st.tile([64, QT], bf16, tag="krt2")
                    nc.vector.tensor_mul(t1, krraw[0:64, :], cosk_sb)
                    nc.vector.tensor_mul(t2, krb, sink_sb)
                    nc.vector.tensor_sub(latk_sh[0:64, 4, :], t1, t2)
                    nc.vector.tensor_mul(t1, krb, cosk_sb)
                    nc.vector.tensor_mul(t2, krraw[0:64, :], sink_sb)
                    nc.vector.tensor_add(krb, t1, t2)  # krb dead; reuse as out
                    nc.sync.dma_start(out=latk_sh[64:128, 4, :], in_=krb)

                    # --- strips for qt0 while the wkv weights stream in ---
                    strips(0, xq0_t)

                    # --- latkv (weights streamed per strip) + allgather ---
                    for s in range(NC_KV):
                        wkv_t = a1w.tile([128, NC_DM * 128], bf16, tag="wkv")
                        nc.sync.dma_start(out=wkv_t, in_=wkv_down.ap()[s])
                        ps = a1ps.tile([128, QT], f32, tag="ps")
                        for c in range(NC_DM):
                            nc.tensor.matmul(
                                ps, wkv_t[:, c * 128:(c + 1) * 128],
                                xk_t[:, c, :], start=(c == 0), stop=(c == NC_DM - 1))
                        nc.scalar.copy(out=latk_sh[:, s, :], in_=ps)
                    nc.sync.dma_start(
                        out=latk_sh_d.ap().rearrange("p (s k) -> p s k", k=QT),
                        in_=latk_sh)
                    nc.gpsimd.collective_compute(
                        "AllGather", mybir.AluOpType.bypass, replica_groups=G_BATCH,
                        ins=[latk_sh_d.ap()], outs=[latk_g_d.ap()])

                # xstream bufs=3: qt3's slot is qt0's buffer (freed early),
                # so all three remaining x tiles prefetch back-to-back
                xq1_t = load_xq(1)
                xq2_t = load_xq(2)
                xq3_t = load_xq(3)
                strips(1, xq1_t)
                strips(2, xq2_t)

                # ----- Phase A3 (k_proj + V) slotted BEFORE the last strip
                # tile: gives qt3's x another ~35us of DMA slack, and the
                # gathered latkv has long arrived by now.
                with tc.tile_pool(name="a3p", bufs=1) as a3p, \
                     tc.tile_pool(name="a3ps", bufs=2, space="PSUM") as a3ps:
                    wku_sb = a3p.tile([128, 4 * NC_KV * 128], bf16)
                    for s in range(4):
                        nc.sync.dma_start(
                            out=wku_sb[:, s * NC_KV * 128:(s + 1) * NC_KV * 128],
                            in_=wk_up.ap()[s])
                    wvu_sb = a3p.tile([128, NC_KV * 512], bf16)
                    nc.sync.dma_start(out=wvu_sb, in_=wv_up.ap())
                    latkv_a = a3p.tile([128, 4, 5, QT], bf16)
                    for kt in range(4):
                        nc.sync.dma_start(
                            out=latkv_a[:, kt],
                            in_=latk_g_d.ap()[kt].rearrange("p (s k) -> p s k", k=QT))
                        nc.vector.tensor_copy(
                            out=krope_sb[:, kt * QT:(kt + 1) * QT],
                            in_=latkv_a[:, kt, 4, :])
                    for kt in range(4):
                        latkv_t = latkv_a[:, kt]
                        for s in range(4):
                            ps = a3ps.tile([128, QT], f32, tag="ps")
                            for c in range(NC_KV):
                                nc.tensor.matmul(
                                    ps, wku_sb[:, (s * NC_KV + c) * 128:(s * NC_KV + c + 1) * 128],
                                    latkv_t[:, c, :], start=(c == 0), stop=(c == NC_KV - 1))
                            # kproj evacuations on ACT: the DVE queue is
                            # backlogged with strip-rope work at this boundary
                            nc.scalar.copy(
                                out=kproj_sb[:, s, kt * QT:(kt + 1) * QT], in_=ps)
                        for kc in range(4):
                            ps = a3ps.tile([128, 512], f32, tag="ps")
                            for c in range(NC_KV):
                                nc.tensor.matmul(
                                    ps, latkv_t[:, c, kc * 128:(kc + 1) * 128],
                                    wvu_sb[:, c * 512:(c + 1) * 512],
                                    start=(c == 0), stop=(c == NC_KV - 1))
                            nc.scalar.copy(out=v_sb[:, kt * 4 + kc, :], in_=ps)

                strips(3, xq3_t)

            # ------------- Phase B: attention + output projection ----------
            with tc.tile_pool(name="bw", bufs=1) as bw, \
                 tc.tile_pool(name="be", bufs=6) as be, \
                 tc.tile_pool(name="ba", bufs=2) as ba, \
                 tc.tile_pool(name="bo", bufs=4) as bo, \
                 tc.tile_pool(name="bps", bufs=2, space="PSUM") as bps, \
                 tc.tile_pool(name="bpd", bufs=2, space="PSUM") as bpd, \
                 tc.tile_pool(name="bpv", bufs=2, space="PSUM") as bpv, \
                 tc.tile_pool(name="bpo", bufs=2, space="PSUM") as bpo:
                wout_sb = bw.tile([128, 64 * 128], bf16)
                nc.sync.dma_start(out=wout_sb, in_=wout.ap())
                mtri_sb = bw.tile([128, 128], bf16)
                nc.sync.dma_start(out=mtri_sb, in_=masktri.ap())
                ones_sb = bw.tile([128, 128], bf16)
                nc.sync.dma_start(out=ones_sb, in_=ones.ap())
                bias_sb = bw.tile([128, NC_DM], f32)
                nc.sync.dma_start(out=bias_sb, in_=bias.ap())

                for qt in range(NQT):
                    q0 = qt * QT
                    K = (q0 + QT) // 128  # causal: chunks 0..K-1
                    attn = ba.tile([128, 4, QT], bf16, tag="attn")
                    for h in range(4):
                        psd = bpd.tile([128, QT], f32, tag="psd")
                        psv = bpv.tile([128, QT], f32, tag="psv")
                        # software-pipelined by one chunk: psd/psv for chunk
                        # kc-1 issue after the scores of kc, so the exp has a
                        # matmul-pair of cover and the PE never waits on ACT
                        pend = None  # (ex, w0) of the previous chunk

                        def flush(last):
                            ex, w0, kc0 = pend
                            nc.tensor.matmul(
                                psd[:, w0:], ones_sb, ex[:, w0:],
                                start=(kc0 == 0), stop=last,
                                skip_group_check=True)
                            nc.tensor.matmul(
                                psv[:, w0:], v_sb[:, kc0, h * 128:(h + 1) * 128],
                                ex[:, w0:],
                                start=(kc0 == 0), stop=last,
                                skip_group_check=True)

                        for kc in range(K):
                            # diagonal chunks: queries before 128*o are fully
                            # masked -- compute only columns [w0, QT)
                            o = kc - q0 // 128
                            w0 = 128 * o if o > 0 else 0
                            pss = bps.tile([128, QT], f32, tag="pss")
                            nc.tensor.matmul(
                                pss[:, w0:], kproj_sb[:, h, kc * 128:(kc + 1) * 128],
                                qn_sb[:, 2 * h, q0 + w0:q0 + QT],
                                start=True, stop=False)
                            nc.tensor.matmul(
                                pss[:, w0:], krope_sb[:, kc * 128:(kc + 1) * 128],
                                qn_sb[:, 2 * h + 1, q0 + w0:q0 + QT],
                                start=False, stop=True)
                            if pend is not None:
                                flush(False)
                            ex = be.tile([128, QT], bf16, tag="ex")
                            nc.scalar.activation(out=ex[:, w0:], in_=pss[:, w0:],
                                                 func=AF.Exp, scale=float(SCALE))
                            if o >= 0:  # triangular mask on the 128-col band
                                nc.vector.tensor_mul(
                                    ex[:, w0:w0 + 128], ex[:, w0:w0 + 128], mtri_sb)
                            pend = (ex, w0, kc)
                        flush(True)
                        rec = be.tile([128, QT], f32, tag="rec")
                        nc.vector.reciprocal_approx_fast(out=rec, in_=psd)
                        nc.vector.tensor_mul(attn[:, h, :], psv, rec)
                    # output projection for this q tile
                    for m in range(NC_DM):
                        pso = bpo.tile([128, QT], f32, tag="pso")
                        for h in range(4):
                            nc.tensor.matmul(
                                pso, wout_sb[:, (m * 4 + h) * 128:(m * 4 + h + 1) * 128],
                                attn[:, h, :], start=(h == 0), stop=(h == 3))
                        oc = bo.tile([128, QT], f32, tag="oc")
                        # alternate evacuation engine so PSUM banks recycle
                        # fast enough to keep the PE from micro-stalling
                        if m % 2 == 0:
                            nc.vector.tensor_scalar_add(
                                oc, pso, bias_sb[:, m:m + 1])
                        else:
                            nc.scalar.activation(
                                out=oc, in_=pso, func=AF.Identity,
                                bias=bias_sb[:, m:m + 1], scale=1.0)
                        nc.sync.dma_start(
                            out=outT.ap()[m * 128:(m + 1) * 128, q0:q0 + QT], in_=oc)

    nc.finalize()
    return nc


def _host_pack(inputs):
    """Build the 8 per-core input maps from the full inputs."""
    import ml_dtypes
    bf16 = ml_dtypes.bfloat16

    xq = np.ascontiguousarray(inputs["inputs_q"], dtype=np.float32)
    xk = np.ascontiguousarray(inputs["inputs_k"], dtype=np.float32)
    Wq_down = np.asarray(inputs["Wq_down"], dtype=np.float32)
    Wkv_down = np.asarray(inputs["Wkv_down"], dtype=np.float32)
    Wq_up = np.asarray(inputs["Wq_up"], dtype=np.float32)
    Wk_up = np.asarray(inputs["Wk_up"], dtype=np.float32)
    Wv_up = np.asarray(inputs["Wv_up"], dtype=np.float32)
    Wq_rope = np.asarray(inputs["Wq_rope"], dtype=np.float32)
    Wk_rope = np.asarray(inputs["Wk_rope"], dtype=np.float32)
    Wout = np.asarray(inputs["Wout"], dtype=np.float32)
    bout = np.asarray(inputs["bout"], dtype=np.float32)

    def pack_lhs(W, n_strips, strip_starts, nchunks):
        out = np.empty((n_strips, 128, nchunks * 128), dtype=bf16)
        for s in range(n_strips):
            blk = W[:, strip_starts[s]:strip_starts[s] + 128]
            out[s] = blk.reshape(nchunks, 128, 128).transpose(1, 0, 2).reshape(128, -1).astype(bf16)
        return out

    # per-512-token-tile, per-partition-contiguous packing:
    # xq_t4[qt, p, c*QT+k] = xq[b, qt*QT+k, c*128+p]
    xq_t4 = [
        np.ascontiguousarray(
            xq[b].T.astype(bf16).reshape(NC_DM, 128, NQT, QT)
            .transpose(2, 1, 0, 3).reshape(NQT, 128, NC_DM * QT))
        for b in range(B)]
    xkT = [xk[b].T.astype(bf16) for b in range(B)]

    # host fold (fp32): full Weff for both kinds, sliced per group below
    weff_a_full = Wq_down @ Wq_up       # [D_MODEL, D_MODEL]
    weff_b_full = Wq_down @ Wq_rope     # [D_MODEL, D_MODEL]

    wkv_down_p = pack_lhs(Wkv_down, NC_KV, [128 * s for s in range(NC_KV)], NC_DM)
    wk_rope_p = pack_lhs(Wk_rope, 1, [0], NC_DM)[0]

    iq = np.arange(1024, dtype=np.float64)
    inv_q = 1.0 / (10000.0 ** (iq * 2.0 / D_MODEL))
    pos = np.arange(S, dtype=np.float64)
    ang_q = pos[:, None] * inv_q[None, :]          # [S, 1024]
    ik = np.arange(64, dtype=np.float64)
    inv_k = 1.0 / (10000.0 ** (ik * 2.0 / HD))
    ang_k = pos[:, None] * inv_k[None, :]          # [S, 64]
    cos_k_full = np.cos(ang_k).T.astype(bf16)  # [64, S]
    sin_k_full = np.sin(ang_k).T.astype(bf16)

    kl = np.arange(128)[:, None]
    ql = np.arange(128)[None, :]
    masktri = np.ascontiguousarray((kl <= ql).astype(np.float32).astype(bf16))
    ones = np.ones((128, 128), dtype=bf16)

    in_maps = []
    for c in range(8):
        b, g = divmod(c, 4)
        cols = _strip_cols(g)
        cols4 = np.concatenate([np.arange(cs, cs + 128) for cs in cols])

        # folded q-weights: [kind][p=dm within chunk][c*512 + s*128 + f]
        weff_p = np.empty((2, 128, NC_DM * 512), dtype=bf16)
        for kind, Wf in ((0, weff_a_full), (1, weff_b_full)):
            Wg = Wf[:, cols4]  # [D_MODEL, 512]
            weff_p[kind] = (
                Wg.reshape(NC_DM, 128, 512).transpose(1, 0, 2)
                .reshape(128, -1).astype(bf16))
        wk_up_p = pack_lhs(Wk_up, 4, cols, NC_KV)
        Wv_g = Wv_up[:, cols4]                      # [512, 512]
        wv_up_p = np.ascontiguousarray(
            Wv_g.reshape(NC_KV, 128, 512).transpose(1, 0, 2).reshape(128, -1).astype(bf16))
        Wout_g = Wout[cols4, :].reshape(4, 128, NC_DM, 128)   # [h][p][m][f]
        wout_p = np.ascontiguousarray(
            Wout_g.transpose(1, 2, 0, 3).reshape(128, -1).astype(bf16))
        cos_q_p = np.empty((2, 128, S), dtype=bf16)
        sin_q_p = np.empty((2, 128, S), dtype=bf16)
        for j in range(2):
            idx = 256 * g + 128 * j + np.arange(128)
            cos_q_p[j] = np.cos(ang_q[:, idx]).T.astype(bf16)
            sin_q_p[j] = np.sin(ang_q[:, idx]).T.astype(bf16)
        bias_p = (bout if g == 0 else np.zeros_like(bout)).reshape(NC_DM, 128)
        bias_p = np.ascontiguousarray(bias_p.T)     # [128, m]

        k0 = QT * g
        xk_p = np.ascontiguousarray(
            xkT[b][:, k0:k0 + QT].reshape(NC_DM, 128, QT)
            .transpose(1, 0, 2).reshape(128, NC_DM * QT))
        in_maps.append({
            "xq_t4": xq_t4[b],
            "xk_p": xk_p,
            "wkv_down": wkv_down_p, "wk_rope": wk_rope_p,
            "weff": weff_p,
            "wk_up": wk_up_p, "wv_up": wv_up_p, "wout": wout_p,
            "cos_q": cos_q_p, "sin_q": sin_q_p,
            "cos_k": np.ascontiguousarray(cos_k_full[:, k0:k0 + QT]),
            "sin_k": np.ascontiguousarray(sin_k_full[:, k0:k0 + QT]),
            "masktri": masktri, "ones": ones, "bias": bias_p,
        })
    return in_maps


def kernel(**inputs):
    global LAST_RESULT
    from concourse.bass_utils import run_bass_kernel_spmd

    if "nc" not in _CACHE:
        _CACHE["nc"] = _build_bass()
    nc = _CACHE["nc"]

    in_maps = _host_pack(inputs)
    kwargs = {}
    if os.environ.get("KERNEL_TRACE"):
        try:
            sys.path.insert(0, os.path.dirname(os.path.abspath(__file__)))
            import axon_shim
            axon_shim.install()
        except Exception:
            pass
        kwargs["trace"] = True
    res = run_bass_kernel_spmd(nc, in_maps, core_ids=list(range(8)), **kwargs)
    LAST_RESULT = res

    out = np.empty((B, S, D_MODEL), dtype=np.float32)
    for b in range(B):
        acc = res.results[4 * b]["outT"].copy()
        for g in range(1, 4):
            acc += res.results[4 * b + g]["outT"]
        out[b] = acc.T
    return out
